# revision 41
# baseline (speedup 1.0000x reference)
"""Trainium2 Bass kernel for nn_AttentionHead (B=4, N=2048, d_model=1024, d_k=64).

Sharding: data-parallel over (batch, query-half) -> 8 cores. Each core gets
q^T[b, :, h*1024:(h+1)*1024], full k^T[b], v^T[b] (host pre-transposes so
d_model lands on SBUF partitions; projections contract d_model), plus the
packed projection weights. All matmuls are bf16 with fp32 PSUM accumulation.

Per-core device program (single pass, engines overlapped):
  1. k/q projections ride a dual-queue (SP+ACT HWDGE) DMA stream of kT/qT
     granules; per-chunk psum accumulators write back k_^T[64,2048] and
     q_^T[64,1024] (zero-padded to 128 partitions), biases folded in.
  2. t-major attention loop over 16 key tiles: scores^T tile
     [128,1024] = k-slice^T q_^T in PSUM, one Exp ACTIVATE per tile
     (scale=1/sqrt(dk) folded in) into a persistent e buffer. The v chain
     (chunk-major vT DMA granules -> projection -> bf16 PE transpose into
     v_aug[nk,65] with a ones column) and the out-matmul accumulation
     out_aug^T[65,512] += v_aug_t^T e_t are hand-interleaved into the PE
     instruction stream. The ones column makes row 64 the softmax
     denominator (unstabilized exp-softmax, faithful to the reference).
  3. Normalize in transposed layout: reciprocal of the denominator row,
     broadcast across partitions with a K=1 matmul, fp32 multiply; the
     [64, 1024] result is DMA'd out and de-transposed on the host.

A small legalization pass hoists excess per-instruction semaphore waits
onto same-engine NoOps (this container's walrus accepts at most one).
"""

import numpy as np
import ml_dtypes

import concourse.bass as bass
import concourse.tile as tile
from concourse import mybir
from concourse.bass_utils import run_bass_kernel_spmd
from concourse.masks import make_identity

B, N, DM, DK = 4, 2048, 1024, 64
NCORES = 8
NQ = N // 2          # queries per core
NK = N               # keys per core
P = 128
NDM = DM // P        # 8 d_model tiles
NKT = NK // P        # 16 key tiles
NQC = 512            # query chunk (one PSUM bank)
NQCH = NQ // NQC     # 2 query chunks
DT = mybir.dt.bfloat16
F32 = mybir.dt.float32
BF = ml_dtypes.bfloat16


# --- walrus wait legalization -------------------------------------------------
# The walrus build in this container accepts at most 1 sync wait + 1 sync
# update per instruction (2 for EventSemaphore). Excess WAITS are hoisted
# onto same-engine NoOps placed just before (queues issue in order, so the
# gating is preserved). Updates are completion-signals and stay put.

def _caps(inst):
    opcode = type(inst).__name__
    if opcode == "InstEventSemaphore":
        return 2, 2
    return 1, 1


def _legalize_waits(nc):
    for f in nc.m.functions:
        for bb in f.blocks:
            out = []
            changed = False
            for inst in bb.instructions:
                si = inst.sync_info
                waits = list(si.on_wait) if si is not None else []
                updates = list(si.on_update) if si is not None else []
                wcap, ucap = _caps(inst)
                if len(waits) <= wcap and len(updates) <= ucap:
                    out.append(inst)
                    continue
                changed = True
                keep_w = waits[len(waits) - wcap:] if wcap else []
                extra_w = waits[: len(waits) - wcap] if wcap else waits
                # Updates signal instruction COMPLETION (writes landed);
                # a following NoOp fires at issue time instead, which races
                # consumers against in-flight writes. Never hoist them.
                assert len(updates) <= ucap, (
                    f"{inst.name}: {len(updates)} sync updates exceed the "
                    f"per-instruction cap and cannot be hoisted safely"
                )
                keep_u = updates
                extra_u = []
                for w in extra_w:
                    nop = mybir.InstNoOp(
                        name=nc.get_next_instruction_name(), ins=[], outs=[]
                    )
                    nop.engine = inst.engine
                    nop.sync_info = mybir.SyncInfo(on_wait=[w], on_update=[])
                    out.append(nop)
                inst.sync_info = mybir.SyncInfo(on_wait=keep_w, on_update=keep_u)
                out.append(inst)
                for u in extra_u:
                    nop = mybir.InstNoOp(
                        name=nc.get_next_instruction_name(), ins=[], outs=[]
                    )
                    nop.engine = inst.engine
                    nop.sync_info = mybir.SyncInfo(on_wait=[], on_update=[u])
                    out.append(nop)
            if changed:
                bb.instructions = out


# --- device program -----------------------------------------------------------

def _build(reps=1):
    nc = bass.Bass()
    qT_d = nc.dram_tensor("qT", [DM, NQ], DT, kind="ExternalInput")
    kT_d = nc.dram_tensor("kT", [DM, NK], DT, kind="ExternalInput")
    vT_d = nc.dram_tensor("vT", [DM, NK], DT, kind="ExternalInput")
    w3_d = nc.dram_tensor("w3", [P, NDM * 3 * DK], DT, kind="ExternalInput")
    b3_d = nc.dram_tensor("b3", [DK, 3], F32, kind="ExternalInput")
    out_d = nc.dram_tensor("out", [DK, NQ], F32, kind="ExternalOutput")

    NCH_K = NK // NQC   # 4 key chunks
    NCH_Q = NQ // NQC   # 2 query chunks
    EXP = mybir.ActivationFunctionType.Exp
    IDF = mybir.ActivationFunctionType.Identity
    SCALE = 1.0 / float(np.sqrt(np.float32(DK)))

    with tile.TileContext(nc) as tc:
      for _rep in range(reps):
        with tc.tile_pool(name="persist", bufs=1) as persist:
            w3_sb = persist.tile([P, NDM, 3 * DK], DT, tag="w3_sb")
            b3_sb = persist.tile([DK, 3], F32, tag="b3_sb")
            ident = persist.tile([P, P], F32, tag="ident")
            identb = persist.tile([P, P], DT, tag="identb")
            k_sbT = persist.tile([P, NK], DT, tag="k_sbT")
            q_sbT = persist.tile([P, NQ], DT, tag="q_sbT")
            v_sbT = persist.tile([P, NK], DT, tag="v_sbT")
            v_aug = persist.tile([P, NKT, DK + 1], DT, tag="v_aug")
            e_all = persist.tile([P, NKT, NQ], DT, tag="e_all")
            out_sbT = persist.tile([DK, NQ], F32, tag="out_sbT")
            onesr = persist.tile([1, DK], DT, tag="onesr")

            with (
                tc.tile_pool(name="xt", bufs=1) as xtp,
                tc.tile_pool(name="psout", bufs=1, space="PSUM") as pso,
            ):
                oacc = [
                    pso.tile([DK + 1, NQC], F32, tag=f"oacc{h}", name=f"oacc{h}")
                    for h in range(NCH_Q)
                ]
                # kq DMA stream: kt0, w3, qt0, kt1, qt1, kt2, kt3, b3
                kts, qts = [], []
                KT_GRAN = [(0, 3), (3, 3), (6, 1), (7, 1)]  # (dmt0, n_dmt)
                def dma_kt(i, eng):
                    d0, nd = KT_GRAN[i]
                    t_ = xtp.tile([P, nd, NK], DT, tag=f"kt{i}", name=f"kt{i}")
                    eng.dma_start(
                        t_[:], kT_d[d0 * P:(d0 + nd) * P, :].rearrange(
                            "(o p) n -> p o n", p=P))
                    kts.append(t_)
                def dma_qt(i, eng):
                    t_ = xtp.tile([P, 4, NQ], DT, tag=f"qt{i}", name=f"qt{i}")
                    eng.dma_start(
                        t_[:], qT_d[i * 4 * P:(i + 1) * 4 * P, :].rearrange(
                            "(o p) n -> p o n", p=P))
                    qts.append(t_)
                # alternate SP/ACT HWDGE queues so per-DMA setup overlaps the
                # serialized transfers (ACT is otherwise idle this early)
                dma_kt(0, nc.sync)
                nc.scalar.dma_start(
                    w3_sb[:], w3_d.rearrange("p (o k) -> p o k", o=NDM))
                nc.scalar.dma_start(b3_sb[:], b3_d[:])
                dma_qt(0, nc.scalar)
                dma_kt(1, nc.sync)
                dma_qt(1, nc.scalar)
                dma_kt(2, nc.sync)
                dma_kt(3, nc.scalar)

                make_identity(nc, ident[:])
                nc.vector.tensor_copy(identb[:], ident[:])
                # preload the exp table set while the DMA stream runs
                nc.scalar.activation(
                    e_all[0:1, 0, 0:1], ident[0:1, 0:1], EXP, scale=1.0)
                nc.gpsimd.memset(k_sbT[DK:P, :], 0.0)
                nc.gpsimd.memset(q_sbT[DK:P, :], 0.0)
                nc.gpsimd.memset(v_sbT[DK:P, :], 0.0)
                nc.gpsimd.memset(v_aug[:], 1.0)  # ones col at [:, :, 64]
                nc.gpsimd.memset(onesr[:], 1.0)

                # ---- k/q projections riding the DMA stream ----
                with tc.tile_pool(name="pskq", bufs=1, space="PSUM") as pskq:
                    psq = [pskq.tile([DK, NQC], F32, tag=f"psq{j}", name=f"psq{j}")
                           for j in range(NCH_Q)]
                    psk = [pskq.tile([DK, NQC], F32, tag=f"psk{j}", name=f"psk{j}")
                           for j in range(NCH_K)]

                    def kt_view(dmt):
                        for i, (d0, nd) in enumerate(KT_GRAN):
                            if d0 <= dmt < d0 + nd:
                                return kts[i][:, dmt - d0, :]
                        raise AssertionError(dmt)
                    def kp(dmts):
                        for dmt in dmts:
                            kv = kt_view(dmt)
                            for j in range(NCH_K):
                                nc.tensor.matmul(
                                    psk[j][:], w3_sb[:, dmt, DK:2 * DK],
                                    kv[:, j * NQC:(j + 1) * NQC],
                                    start=(dmt == 0), stop=(dmt == NDM - 1))
                    def qp(dmts):
                        for dmt in dmts:
                            for j in range(NCH_Q):
                                nc.tensor.matmul(
                                    psq[j][:], w3_sb[:, dmt, 0:DK],
                                    qts[dmt // 4][:, dmt % 4,
                                                  j * NQC:(j + 1) * NQC],
                                    start=(dmt == 0), stop=(dmt == NDM - 1))
                    kp([0])
                    kp([1, 2])
                    qp([0, 1, 2, 3])
                    kp([3, 4, 5])
                    # keep the PE warm while qt1/kt granules land; results
                    # are discarded (first real oacc matmul resets the bank)
                    for _w in range(14):
                        nc.tensor.matmul(
                            oacc[0][0:DK, 0:NQC], w3_sb[:, 0, 0:DK],
                            k_sbT[:, NK - NQC:NK], start=True, stop=True)
                    qp([4, 5, 6, 7])
                    kp([6])
                    kp([7])
                    # writebacks: k0,k1 on ACT; q0,q1,k2,k3 on DVE — so the
                    # psum banks reused by the scores pool free earliest
                    nc.scalar.activation(
                        k_sbT[0:DK, 0:NQC], psk[0][:], IDF, bias=b3_sb[:, 1:2])
                    nc.vector.tensor_scalar_add(
                        q_sbT[0:DK, 0:NQC], psq[0][:], b3_sb[:, 0:1])
                    nc.scalar.activation(
                        k_sbT[0:DK, NQC:2 * NQC], psk[1][:], IDF,
                        bias=b3_sb[:, 1:2])
                    nc.vector.tensor_scalar_add(
                        q_sbT[0:DK, NQC:2 * NQC], psq[1][:], b3_sb[:, 0:1])
                    nc.vector.tensor_scalar_add(
                        k_sbT[0:DK, 2 * NQC:3 * NQC], psk[2][:], b3_sb[:, 1:2])
                    nc.vector.tensor_scalar_add(
                        k_sbT[0:DK, 3 * NQC:4 * NQC], psk[3][:], b3_sb[:, 1:2])

                # ---- attention (t-major) with pipelined v chain ----
                # vT loaded chunk-major: granule j = all d_model for keys
                # [j*512, (j+1)*512); its projection, writeback, transposes
                # and the out-matmuls are interleaved into the scores/exp loop.
                vts = []
                for j in range(NCH_K):
                    vt = xtp.tile([P, NDM, NQC], DT, tag=f"vt{j}", name=f"vt{j}")
                    nc.sync.dma_start(
                        vt[:], vT_d[:, j * NQC:(j + 1) * NQC].rearrange(
                            "(o p) n -> p o n", p=P))
                    vts.append(vt)
                with (
                    tc.tile_pool(name="psscore", bufs=2, space="PSUM") as pss,
                    tc.tile_pool(name="psv", bufs=1, space="PSUM") as psvp,
                ):
                    psva = [None]

                    def v_mm(j, dmts):
                        if dmts[0] == 0:
                            psva[0] = psvp.tile(
                                [DK, NQC], F32, tag="psvacc", name=f"psva{j}")
                        for dmt in dmts:
                            nc.tensor.matmul(
                                psva[0][:], w3_sb[:, dmt, 2 * DK:3 * DK],
                                vts[j][:, dmt, :],
                                start=(dmt == 0), stop=(dmt == NDM - 1))
                    def v_wb(j):
                        nc.vector.tensor_scalar_add(
                            v_sbT[0:DK, j * NQC:(j + 1) * NQC], psva[0][:],
                            b3_sb[:, 2:3])
                    def v_tr(ts_):
                        for t_ in ts_:
                            pt = psvp.tile([P, P], DT, tag="psvb", name=f"pvb{t_}")
                            nc.tensor.transpose(
                                pt[:], v_sbT[:, t_ * P:(t_ + 1) * P], identb[:])
                            nc.vector.tensor_copy(v_aug[:, t_, 0:DK],
                                                  pt[:, 0:DK])
                    def o_mm(tp):
                        for h in range(NCH_Q):
                            nc.tensor.matmul(
                                oacc[h][:], v_aug[:, tp, :],
                                e_all[:, tp, h * NQC:(h + 1) * NQC],
                                start=(tp == 0), stop=(tp == NKT - 1))

                    # per-slot v-pipeline work: chunk j MMs at slots 4j+1/4j+2,
                    # writeback after, transposes at 4j+3/4j+4
                    vwork = {}
                    for j in range(NCH_K):
                        vwork.setdefault(2 * j + 1, []).append(
                            lambda j=j: v_mm(j, [0, 1, 2, 3]))
                        vwork.setdefault(2 * j + 2, []).append(
                            lambda j=j: (v_mm(j, [4, 5, 6, 7]), v_wb(j)))
                        vwork.setdefault(2 * j + 3, []).append(
                            lambda j=j: v_tr([4 * j, 4 * j + 1]))
                        vwork.setdefault(2 * j + 4, []).append(
                            lambda j=j: v_tr([4 * j + 2, 4 * j + 3]))

                    ODELAY = 5
                    def emit_scores(t):
                        sc = pss.tile([P, NQ], F32, tag="psscore",
                                      name=f"sc{t}")
                        for h in range(NCH_Q):
                            nc.tensor.matmul(
                                sc[:, h * NQC:(h + 1) * NQC],
                                k_sbT[:, t * P:(t + 1) * P],
                                q_sbT[:, h * NQC:(h + 1) * NQC],
                                start=True, stop=True)
                        return sc
                    # scores run one slot ahead of their exp so the per-slot
                    # v-chain/out-matmul work can never starve the ACT engine
                    sc_cur = emit_scores(0)
                    for t in range(NKT):
                        if t + 1 < NKT:
                            sc_next = emit_scores(t + 1)
                        nc.scalar.activation(
                            e_all[:, t, :], sc_cur[:], EXP, scale=SCALE)
                        if t + 1 < NKT:
                            sc_cur = sc_next
                        for fn in vwork.get(t, []):
                            fn()
                        if t >= ODELAY:
                            o_mm(t - ODELAY)
                    for fn in vwork.get(NKT, []):
                        fn()
                    for tp in range(NKT - ODELAY, NKT):
                        o_mm(tp)

                # ---- normalize in transposed layout + store ----
                # out^T[dk, nq] = oacc[0:64] * (1/oacc[64]) ; the reciprocal
                # row is broadcast across partitions with a K=1 matmul.
                with (
                    tc.tile_pool(name="fin", bufs=2) as fin,
                    tc.tile_pool(name="psfin", bufs=2, space="PSUM") as psf,
                ):
                    for h in range(NCH_Q):
                        rcr = fin.tile([1, NQC], F32, tag="rcr")
                        nc.vector.reciprocal(rcr[:], oacc[h][DK:DK + 1, :])
                        rcb = fin.tile([1, NQC], DT, tag="rcb")
                        nc.vector.tensor_copy(rcb[:], rcr[:])
                        pb = psf.tile([DK, NQC], F32, tag="psfin")
                        nc.tensor.matmul(
                            pb[:], onesr[:], rcb[:], start=True, stop=True)
                        rcf = fin.tile([DK, NQC], F32, tag="rcf")
                        nc.scalar.copy(rcf[:], pb[:])
                        nc.vector.tensor_tensor(
                            out_sbT[:, h * NQC:(h + 1) * NQC],
                            oacc[h][0:DK, :], rcf[:], mybir.AluOpType.mult)
                        (nc.sync if h == 0 else nc.scalar).dma_start(
                            out_d[:, h * NQC:(h + 1) * NQC],
                            out_sbT[:, h * NQC:(h + 1) * NQC])
    _legalize_waits(nc)
    return nc


_nc_cache = None


def _get_nc():
    global _nc_cache
    if _nc_cache is None:
        _nc_cache = _build()
    return _nc_cache


def _marshal(q, k, v, Wq, bq, Wk, bk, Wv, bv):
    """Host-side layout prep: transpose to [B, d_model, N], cast to bf16,
    shard over (batch, query-half)."""
    qT = np.ascontiguousarray(np.transpose(np.asarray(q), (0, 2, 1))).astype(BF)
    kT = np.ascontiguousarray(np.transpose(np.asarray(k), (0, 2, 1))).astype(BF)
    vT = np.ascontiguousarray(np.transpose(np.asarray(v), (0, 2, 1))).astype(BF)
    w3 = np.concatenate(
        [np.asarray(Wq), np.asarray(Wk), np.asarray(Wv)], axis=1
    ).astype(BF)
    # [1024, 192] -> [128, 8*192] partition-major so the DMA is contiguous
    w3 = np.ascontiguousarray(
        w3.reshape(NDM, P, 3 * DK).transpose(1, 0, 2).reshape(P, NDM * 3 * DK)
    )
    b3 = np.stack(
        [np.asarray(bq), np.asarray(bk), np.asarray(bv)], axis=1
    ).astype(np.float32)
    in_maps = []
    for c in range(NCORES):
        bi, h = divmod(c, 2)
        in_maps.append({
            "qT": np.ascontiguousarray(qT[bi][:, h * NQ:(h + 1) * NQ]),
            "kT": kT[bi],
            "vT": vT[bi],
            "w3": w3, "b3": b3,
        })
    return in_maps


def _unmarshal(results):
    out = np.empty((B, N, DK), np.float32)
    for c in range(NCORES):
        bi, h = divmod(c, 2)
        out[bi, h * NQ:(h + 1) * NQ] = results[c]["out"].T
    return out


def kernel(q, k, v, Wq, bq, Wk, bk, Wv, bv):
    in_maps = _marshal(q, k, v, Wq, bq, Wk, bk, Wv, bv)
    res = run_bass_kernel_spmd(_get_nc(), in_maps, core_ids=list(range(NCORES)))
    return _unmarshal(res.results)


# revision 42
# speedup vs baseline: 1.0016x; 1.0016x over previous
"""Trainium2 Bass kernel for nn_AttentionHead (B=4, N=2048, d_model=1024, d_k=64).

Sharding: data-parallel over (batch, query-half) -> 8 cores. Each core gets
q^T[b, :, h*1024:(h+1)*1024], full k^T[b], v^T[b] (host pre-transposes so
d_model lands on SBUF partitions; projections contract d_model), plus the
packed projection weights. All matmuls are bf16 with fp32 PSUM accumulation.

Per-core device program (single pass, engines overlapped):
  1. k/q projections ride a dual-queue (SP+ACT HWDGE) DMA stream of kT/qT
     granules; per-chunk psum accumulators write back k_^T[64,2048] and
     q_^T[64,1024] (zero-padded to 128 partitions), biases folded in.
  2. t-major attention loop over 16 key tiles: scores^T tile
     [128,1024] = k-slice^T q_^T in PSUM, one Exp ACTIVATE per tile
     (scale=1/sqrt(dk) folded in) into a persistent e buffer. The v chain
     (chunk-major vT DMA granules -> projection -> bf16 PE transpose into
     v_aug[nk,65] with a ones column) and the out-matmul accumulation
     out_aug^T[65,512] += v_aug_t^T e_t are hand-interleaved into the PE
     instruction stream. The ones column makes row 64 the softmax
     denominator (unstabilized exp-softmax, faithful to the reference).
  3. Normalize in transposed layout: reciprocal of the denominator row,
     broadcast across partitions with a K=1 matmul, fp32 multiply; the
     [64, 1024] result is DMA'd out and de-transposed on the host.

A small legalization pass hoists excess per-instruction semaphore waits
onto same-engine NoOps (this container's walrus accepts at most one).
"""

import numpy as np
import ml_dtypes

import concourse.bass as bass
import concourse.tile as tile
from concourse import mybir
from concourse.bass_utils import run_bass_kernel_spmd
from concourse.masks import make_identity

B, N, DM, DK = 4, 2048, 1024, 64
NCORES = 8
NQ = N // 2          # queries per core
NK = N               # keys per core
P = 128
NDM = DM // P        # 8 d_model tiles
NKT = NK // P        # 16 key tiles
NQC = 512            # query chunk (one PSUM bank)
NQCH = NQ // NQC     # 2 query chunks
DT = mybir.dt.bfloat16
F32 = mybir.dt.float32
BF = ml_dtypes.bfloat16


# --- walrus wait legalization -------------------------------------------------
# The walrus build in this container accepts at most 1 sync wait + 1 sync
# update per instruction (2 for EventSemaphore). Excess WAITS are hoisted
# onto same-engine NoOps placed just before (queues issue in order, so the
# gating is preserved). Updates are completion-signals and stay put.

def _caps(inst):
    opcode = type(inst).__name__
    if opcode == "InstEventSemaphore":
        return 2, 2
    return 1, 1


def _legalize_waits(nc):
    for f in nc.m.functions:
        for bb in f.blocks:
            out = []
            changed = False
            for inst in bb.instructions:
                si = inst.sync_info
                waits = list(si.on_wait) if si is not None else []
                updates = list(si.on_update) if si is not None else []
                wcap, ucap = _caps(inst)
                if len(waits) <= wcap and len(updates) <= ucap:
                    out.append(inst)
                    continue
                changed = True
                keep_w = waits[len(waits) - wcap:] if wcap else []
                extra_w = waits[: len(waits) - wcap] if wcap else waits
                # Updates signal instruction COMPLETION (writes landed);
                # a following NoOp fires at issue time instead, which races
                # consumers against in-flight writes. Never hoist them.
                assert len(updates) <= ucap, (
                    f"{inst.name}: {len(updates)} sync updates exceed the "
                    f"per-instruction cap and cannot be hoisted safely"
                )
                keep_u = updates
                extra_u = []
                for w in extra_w:
                    nop = mybir.InstNoOp(
                        name=nc.get_next_instruction_name(), ins=[], outs=[]
                    )
                    nop.engine = inst.engine
                    nop.sync_info = mybir.SyncInfo(on_wait=[w], on_update=[])
                    out.append(nop)
                inst.sync_info = mybir.SyncInfo(on_wait=keep_w, on_update=keep_u)
                out.append(inst)
                for u in extra_u:
                    nop = mybir.InstNoOp(
                        name=nc.get_next_instruction_name(), ins=[], outs=[]
                    )
                    nop.engine = inst.engine
                    nop.sync_info = mybir.SyncInfo(on_wait=[], on_update=[u])
                    out.append(nop)
            if changed:
                bb.instructions = out


# --- device program -----------------------------------------------------------

def _build(reps=1):
    nc = bass.Bass()
    qT_d = nc.dram_tensor("qT", [DM, NQ], DT, kind="ExternalInput")
    kT_d = nc.dram_tensor("kT", [DM, NK], DT, kind="ExternalInput")
    vT_d = nc.dram_tensor("vT", [DM, NK], DT, kind="ExternalInput")
    w3_d = nc.dram_tensor("w3", [P, NDM * 3 * DK], DT, kind="ExternalInput")
    b3_d = nc.dram_tensor("b3", [DK, 3], F32, kind="ExternalInput")
    out_d = nc.dram_tensor("out", [DK, NQ], F32, kind="ExternalOutput")

    NCH_K = NK // NQC   # 4 key chunks
    NCH_Q = NQ // NQC   # 2 query chunks
    EXP = mybir.ActivationFunctionType.Exp
    IDF = mybir.ActivationFunctionType.Identity
    SCALE = 1.0 / float(np.sqrt(np.float32(DK)))

    with tile.TileContext(nc) as tc:
      for _rep in range(reps):
        with tc.tile_pool(name="persist", bufs=1) as persist:
            w3_sb = persist.tile([P, NDM, 3 * DK], DT, tag="w3_sb")
            b3_sb = persist.tile([DK, 3], F32, tag="b3_sb")
            ident = persist.tile([P, P], F32, tag="ident")
            identb = persist.tile([P, P], DT, tag="identb")
            k_sbT = persist.tile([P, NK], DT, tag="k_sbT")
            q_sbT = persist.tile([P, NQ], DT, tag="q_sbT")
            v_sbT = persist.tile([P, NK], DT, tag="v_sbT")
            v_aug = persist.tile([P, NKT, DK + 1], DT, tag="v_aug")
            e_all = persist.tile([P, NKT, NQ], DT, tag="e_all")
            out_sbT = persist.tile([DK, NQ], F32, tag="out_sbT")
            onesr = persist.tile([1, DK], DT, tag="onesr")

            with (
                tc.tile_pool(name="xt", bufs=1) as xtp,
                tc.tile_pool(name="psout", bufs=1, space="PSUM") as pso,
            ):
                oacc = [
                    pso.tile([DK + 1, NQC], F32, tag=f"oacc{h}", name=f"oacc{h}")
                    for h in range(NCH_Q)
                ]
                # kq DMA stream: kt0, w3, qt0, kt1, qt1, kt2, kt3, b3
                kts, qts = [], []
                KT_GRAN = [(0, 3), (3, 3), (6, 1), (7, 1)]  # (dmt0, n_dmt)
                def dma_kt(i, eng):
                    d0, nd = KT_GRAN[i]
                    t_ = xtp.tile([P, nd, NK], DT, tag=f"kt{i}", name=f"kt{i}")
                    eng.dma_start(
                        t_[:], kT_d[d0 * P:(d0 + nd) * P, :].rearrange(
                            "(o p) n -> p o n", p=P))
                    kts.append(t_)
                def dma_qt(i, eng):
                    t_ = xtp.tile([P, 4, NQ], DT, tag=f"qt{i}", name=f"qt{i}")
                    eng.dma_start(
                        t_[:], qT_d[i * 4 * P:(i + 1) * 4 * P, :].rearrange(
                            "(o p) n -> p o n", p=P))
                    qts.append(t_)
                # alternate SP/ACT HWDGE queues so per-DMA setup overlaps the
                # serialized transfers (ACT is otherwise idle this early)
                dma_kt(0, nc.sync)
                nc.scalar.dma_start(
                    w3_sb[:], w3_d.rearrange("p (o k) -> p o k", o=NDM))
                nc.scalar.dma_start(b3_sb[:], b3_d[:])
                dma_qt(0, nc.scalar)
                dma_kt(1, nc.sync)
                dma_qt(1, nc.scalar)
                dma_kt(2, nc.sync)
                dma_kt(3, nc.scalar)

                make_identity(nc, ident[:])
                nc.vector.tensor_copy(identb[:], ident[:])
                # preload the exp table set while the DMA stream runs
                nc.scalar.activation(
                    e_all[0:1, 0, 0:1], ident[0:1, 0:1], EXP, scale=1.0)
                nc.gpsimd.memset(k_sbT[DK:P, :], 0.0)
                nc.gpsimd.memset(q_sbT[DK:P, :], 0.0)
                nc.gpsimd.memset(v_sbT[DK:P, :], 0.0)
                nc.gpsimd.memset(v_aug[:], 1.0)  # ones col at [:, :, 64]
                nc.gpsimd.memset(onesr[:], 1.0)

                # ---- k/q projections riding the DMA stream ----
                with tc.tile_pool(name="pskq", bufs=1, space="PSUM") as pskq:
                    psq = [pskq.tile([DK, NQC], F32, tag=f"psq{j}", name=f"psq{j}")
                           for j in range(NCH_Q)]
                    psk = [pskq.tile([DK, NQC], F32, tag=f"psk{j}", name=f"psk{j}")
                           for j in range(NCH_K)]

                    def kt_view(dmt):
                        for i, (d0, nd) in enumerate(KT_GRAN):
                            if d0 <= dmt < d0 + nd:
                                return kts[i][:, dmt - d0, :]
                        raise AssertionError(dmt)
                    def kp(dmts):
                        for dmt in dmts:
                            kv = kt_view(dmt)
                            for j in range(NCH_K):
                                nc.tensor.matmul(
                                    psk[j][:], w3_sb[:, dmt, DK:2 * DK],
                                    kv[:, j * NQC:(j + 1) * NQC],
                                    start=(dmt == 0), stop=(dmt == NDM - 1))
                    def qp(dmts):
                        for dmt in dmts:
                            for j in range(NCH_Q):
                                nc.tensor.matmul(
                                    psq[j][:], w3_sb[:, dmt, 0:DK],
                                    qts[dmt // 4][:, dmt % 4,
                                                  j * NQC:(j + 1) * NQC],
                                    start=(dmt == 0), stop=(dmt == NDM - 1))
                    kp([0])
                    kp([1, 2])
                    qp([0, 1, 2, 3])
                    kp([3, 4, 5])
                    # keep the PE warm while qt1/kt granules land; results
                    # are discarded (first real oacc matmul resets the bank)
                    for _w in range(14):
                        nc.tensor.matmul(
                            oacc[0][0:DK, 0:NQC], w3_sb[:, 0, 0:DK],
                            k_sbT[:, NK - NQC:NK], start=True, stop=True)
                    qp([4, 5, 6, 7])
                    kp([6])
                    kp([7])
                    # writebacks: k0,k1 on ACT; q0,q1,k2,k3 on DVE — so the
                    # psum banks reused by the scores pool free earliest
                    nc.scalar.activation(
                        k_sbT[0:DK, 0:NQC], psk[0][:], IDF, bias=b3_sb[:, 1:2])
                    nc.vector.tensor_scalar_add(
                        q_sbT[0:DK, 0:NQC], psq[0][:], b3_sb[:, 0:1])
                    nc.scalar.activation(
                        k_sbT[0:DK, NQC:2 * NQC], psk[1][:], IDF,
                        bias=b3_sb[:, 1:2])
                    nc.vector.tensor_scalar_add(
                        q_sbT[0:DK, NQC:2 * NQC], psq[1][:], b3_sb[:, 0:1])
                    nc.vector.tensor_scalar_add(
                        k_sbT[0:DK, 2 * NQC:3 * NQC], psk[2][:], b3_sb[:, 1:2])
                    nc.vector.tensor_scalar_add(
                        k_sbT[0:DK, 3 * NQC:4 * NQC], psk[3][:], b3_sb[:, 1:2])

                # ---- attention (t-major) with pipelined v chain ----
                # vT loaded chunk-major: granule j = all d_model for keys
                # [j*512, (j+1)*512); its projection, writeback, transposes
                # and the out-matmuls are interleaved into the scores/exp loop.
                vts = []
                for j in range(NCH_K):
                    vt = xtp.tile([P, NDM, NQC], DT, tag=f"vt{j}", name=f"vt{j}")
                    nc.sync.dma_start(
                        vt[:], vT_d[:, j * NQC:(j + 1) * NQC].rearrange(
                            "(o p) n -> p o n", p=P))
                    vts.append(vt)
                with (
                    tc.tile_pool(name="psscore", bufs=2, space="PSUM") as pss,
                    tc.tile_pool(name="psv", bufs=1, space="PSUM") as psvp,
                ):
                    psva = [None]

                    def v_mm(j, dmts):
                        if dmts[0] == 0:
                            psva[0] = psvp.tile(
                                [DK, NQC], F32, tag="psvacc", name=f"psva{j}")
                        for dmt in dmts:
                            nc.tensor.matmul(
                                psva[0][:], w3_sb[:, dmt, 2 * DK:3 * DK],
                                vts[j][:, dmt, :],
                                start=(dmt == 0), stop=(dmt == NDM - 1))
                    def v_wb(j):
                        nc.vector.tensor_scalar_add(
                            v_sbT[0:DK, j * NQC:(j + 1) * NQC], psva[0][:],
                            b3_sb[:, 2:3])
                    def v_tr(ts_):
                        for t_ in ts_:
                            pt = psvp.tile([P, P], DT, tag="psvb", name=f"pvb{t_}")
                            nc.tensor.transpose(
                                pt[:], v_sbT[:, t_ * P:(t_ + 1) * P], identb[:])
                            nc.vector.tensor_copy(v_aug[:, t_, 0:DK],
                                                  pt[:, 0:DK])
                    def o_mm(tp):
                        for h in range(NCH_Q):
                            nc.tensor.matmul(
                                oacc[h][:], v_aug[:, tp, :],
                                e_all[:, tp, h * NQC:(h + 1) * NQC],
                                start=(tp == 0), stop=(tp == NKT - 1))

                    # per-slot v-pipeline work: chunk j MMs at slots 4j+1/4j+2,
                    # writeback after, transposes at 4j+3/4j+4
                    vwork = {}
                    for j in range(NCH_K):
                        vwork.setdefault(2 * j + 1, []).append(
                            lambda j=j: v_mm(j, [0, 1, 2, 3]))
                        vwork.setdefault(2 * j + 2, []).append(
                            lambda j=j: (v_mm(j, [4, 5, 6, 7]), v_wb(j)))
                        vwork.setdefault(2 * j + 3, []).append(
                            lambda j=j: v_tr([4 * j, 4 * j + 1]))
                        vwork.setdefault(2 * j + 4, []).append(
                            lambda j=j: v_tr([4 * j + 2, 4 * j + 3]))

                    ODELAY = 5
                    def emit_scores(t):
                        sc = pss.tile([P, NQ], F32, tag="psscore",
                                      name=f"sc{t}")
                        for h in range(NCH_Q):
                            nc.tensor.matmul(
                                sc[:, h * NQC:(h + 1) * NQC],
                                k_sbT[:, t * P:(t + 1) * P],
                                q_sbT[:, h * NQC:(h + 1) * NQC],
                                start=True, stop=True)
                        return sc
                    # scores run one slot ahead of their exp so the per-slot
                    # v-chain/out-matmul work can never starve the ACT engine
                    sc_cur = emit_scores(0)
                    for t in range(NKT):
                        if t + 1 < NKT:
                            sc_next = emit_scores(t + 1)
                        nc.scalar.activation(
                            e_all[:, t, :], sc_cur[:], EXP, scale=SCALE)
                        if t + 1 < NKT:
                            sc_cur = sc_next
                        for fn in vwork.get(t, []):
                            fn()
                        if t >= ODELAY:
                            o_mm(t - ODELAY)
                    for fn in vwork.get(NKT, []):
                        fn()
                    for tp in range(NKT - ODELAY, NKT):
                        o_mm(tp)

                # ---- normalize in transposed layout + store ----
                # out^T[dk, nq] = oacc[0:64] * (1/oacc[64]) ; the reciprocal
                # row is broadcast across partitions with a K=1 matmul.
                with (
                    tc.tile_pool(name="fin", bufs=2) as fin,
                    tc.tile_pool(name="psfin", bufs=2, space="PSUM") as psf,
                ):
                    for h in range(NCH_Q):
                        rcr = fin.tile([1, NQC], F32, tag="rcr")
                        nc.vector.reciprocal(rcr[:], oacc[h][DK:DK + 1, :])
                        rcb = fin.tile([1, NQC], DT, tag="rcb")
                        nc.scalar.copy(rcb[:], rcr[:])
                        pb = psf.tile([DK, NQC], F32, tag="psfin")
                        nc.tensor.matmul(
                            pb[:], onesr[:], rcb[:], start=True, stop=True)
                        rcf = fin.tile([DK, NQC], F32, tag="rcf")
                        nc.scalar.copy(rcf[:], pb[:])
                        nc.vector.tensor_tensor(
                            out_sbT[:, h * NQC:(h + 1) * NQC],
                            oacc[h][0:DK, :], rcf[:], mybir.AluOpType.mult)
                        (nc.sync if h == 0 else nc.scalar).dma_start(
                            out_d[:, h * NQC:(h + 1) * NQC],
                            out_sbT[:, h * NQC:(h + 1) * NQC])
    _legalize_waits(nc)
    return nc


_nc_cache = None


def _get_nc():
    global _nc_cache
    if _nc_cache is None:
        _nc_cache = _build()
    return _nc_cache


def _marshal(q, k, v, Wq, bq, Wk, bk, Wv, bv):
    """Host-side layout prep: transpose to [B, d_model, N], cast to bf16,
    shard over (batch, query-half)."""
    qT = np.ascontiguousarray(np.transpose(np.asarray(q), (0, 2, 1))).astype(BF)
    kT = np.ascontiguousarray(np.transpose(np.asarray(k), (0, 2, 1))).astype(BF)
    vT = np.ascontiguousarray(np.transpose(np.asarray(v), (0, 2, 1))).astype(BF)
    w3 = np.concatenate(
        [np.asarray(Wq), np.asarray(Wk), np.asarray(Wv)], axis=1
    ).astype(BF)
    # [1024, 192] -> [128, 8*192] partition-major so the DMA is contiguous
    w3 = np.ascontiguousarray(
        w3.reshape(NDM, P, 3 * DK).transpose(1, 0, 2).reshape(P, NDM * 3 * DK)
    )
    b3 = np.stack(
        [np.asarray(bq), np.asarray(bk), np.asarray(bv)], axis=1
    ).astype(np.float32)
    in_maps = []
    for c in range(NCORES):
        bi, h = divmod(c, 2)
        in_maps.append({
            "qT": np.ascontiguousarray(qT[bi][:, h * NQ:(h + 1) * NQ]),
            "kT": kT[bi],
            "vT": vT[bi],
            "w3": w3, "b3": b3,
        })
    return in_maps


def _unmarshal(results):
    out = np.empty((B, N, DK), np.float32)
    for c in range(NCORES):
        bi, h = divmod(c, 2)
        out[bi, h * NQ:(h + 1) * NQ] = results[c]["out"].T
    return out


def kernel(q, k, v, Wq, bq, Wk, bk, Wv, bv):
    in_maps = _marshal(q, k, v, Wq, bq, Wk, bk, Wv, bv)
    res = run_bass_kernel_spmd(_get_nc(), in_maps, core_ids=list(range(NCORES)))
    return _unmarshal(res.results)
